# revision 28
# baseline (speedup 1.0000x reference)
"""ResNet BasicBlock (conv3x3-BN-ReLU-conv3x3-BN-add-ReLU) on 8 Trainium2 cores.

Data-parallel over batch: 32 samples -> 4 per core. Each 3x3 conv runs as a
Winograd F(2,3) transform along H (1.5x fewer PE MACs than direct conv):
rows are pre-combined on the GpSimd/Vector engines into 4 j-planes
(V0=r0-r2, V1=r1+r2, V2=r2-r1, V3=r1-r3, fp16), the PE accumulates
M_j = sum_{kw,ci} U_j[kw,ci]^T V_j(shifted kw) into 4 PSUM banks, and the
Vector engine folds the inverse transform (y_even = M0+M1+M2,
y_odd = M1-M2-M3) with one bank-spanning reduce + one scalar_tensor_tensor
per output row-pair chunk. BN scale is folded into the Winograd weights on
host; bias + ReLU run on the Scalar engine; the residual add runs on GpSimd.
V planes live in per-chunk tiles so Tile's whole-tile dependency tracking
stays chunk-granular (a matmul only waits for the transform wave it reads).
Images are zero-padded to 58x58 on the host so SBUF loads are contiguous.
"""
import os
import sys

for _p in ("/opt/trn_rl_repo", "/root/.axon_site/_ro/trn_rl_repo"):
    if os.path.isdir(_p) and _p not in sys.path:
        sys.path.append(_p)

import numpy as np

EPS = 1e-5

S = 4            # samples per core
C = 256
H = W = 56
PH = 58          # padded rows (img rows -1..56)
PW = 60          # row pitch: image cols at 2..57, ring zeros at cols 1,58
FLAT = PH * PW   # 3480
T = 28           # winograd row-tiles per image (2 output rows each)
TCH = 7          # tiles per chunk
NCH = 4          # chunks (4*7 = 28 tiles)
NROW = TCH * W   # 392 moving rows per matmul
VW = 58          # V plane cols (xpad cols 1..58)
VCH = 4 * TCH * VW  # V chunk-tile free size: [4j, 7t, 58]
N_CORES = 8

_CACHE = {}
LAST_RESULT = None


def _build():
    from concourse import bacc
    import concourse.mybir as mybir
    import concourse.tile as tile

    F32 = mybir.dt.float32
    F16 = mybir.dt.float16
    Relu = mybir.ActivationFunctionType.Relu
    Alu = mybir.AluOpType
    AxX = mybir.AxisListType.X

    nc = bacc.Bacc(None, target_bir_lowering=False)

    x_d = nc.dram_tensor("x", [S, C, PH, PW], F16, kind="ExternalInput")
    u1_d = nc.dram_tensor("u1t", [2, 128, 24, 128], F16, kind="ExternalInput")
    u2_d = nc.dram_tensor("u2t", [2, 128, 24, 128], F16, kind="ExternalInput")
    b1_d = nc.dram_tensor("b1t", [128, 2], F32, kind="ExternalInput")
    b2_d = nc.dram_tensor("b2t", [128, 2], F32, kind="ExternalInput")
    z_d = nc.dram_tensor("zeros", [128, FLAT], F16, kind="ExternalInput")
    y_d = nc.dram_tensor("y", [S, C, H, W], F32, kind="ExternalOutput")

    with tile.TileContext(nc) as tc:
        with (
            tc.tile_pool(name="wpool", bufs=1) as wpool,
            tc.tile_pool(name="img", bufs=1) as img,
            tc.tile_pool(name="ep", bufs=3) as ep,
            tc.tile_pool(name="yp", bufs=3) as yp,
            tc.tile_pool(name="ps", bufs=2, space="PSUM") as ps,
        ):
            u_sb = {}
            for conv in (1, 2):
                for ci in range(2):
                    u_sb[(conv, ci)] = wpool.tile(
                        [128, 24 * 128], F16, name=f"u{conv}_{ci}")
            b1_t = wpool.tile([128, 2], F32, name="b1_t")
            b2_t = wpool.tile([128, 2], F32, name="b2_t")

            xpad = {}
            o1pad = {}
            vx = {}   # (b, ci, c) -> [128, 4, 7, 58] chunk tile
            vo = {}   # (ci, c)    -> [128, 4, 7, 58] chunk tile
            # xpad is triple-buffered (keyed s%3): load_sample(s+2) must not
            # WAR-depend on conv2(s)'s residual reads, or the scheduler's
            # hoisted V(x) transforms deadlock the DVE queue against it.
            for b in range(3):
                for ci in range(2):
                    xpad[(b, ci)] = img.tile([128, FLAT], F16, name=f"xpad{b}_{ci}")
            for b in range(2):
                for ci in range(2):
                    o1pad[(b, ci)] = img.tile([128, FLAT], F16, name=f"o1pad{b}_{ci}")
                    for c in range(NCH):
                        vx[(b, ci, c)] = img.tile(
                            [128, VCH], F16, name=f"vx{b}_{ci}_{c}")
            for ci in range(2):
                for c in range(NCH):
                    vo[(ci, c)] = img.tile([128, VCH], F16, name=f"vo_{ci}_{c}")

            def view(t):
                return t.rearrange("p (h w) -> p h w", h=PH)

            def view2(t):
                # [p, 29, 2, 60]: row 2t+a at [:, t, a, :]
                return t.rearrange("p (t two w) -> p t two w", two=2, w=PW)

            def vview(t):
                return t.rearrange("p (j t w) -> p j t w", j=4, t=TCH)

            def load_weights(conv, ud, blks=(0, 24)):
                k0, k1 = blks
                for ci in range(2):
                    nc.sync.dma_start(
                        u_sb[(conv, ci)][:, k0 * 128:k1 * 128],
                        ud[ci, :, k0:k1, :].rearrange("p a b -> p (a b)"))

            def load_sample(s, bands=((0, PH),)):
                b = s % 3
                for r0, r1 in bands:
                    for ci in range(2):
                        nc.sync.dma_start(
                            view(xpad[(b, ci)])[:, r0:r1, :],
                            x_d[s, ci * 128:(ci + 1) * 128, r0:r1, :])

            def zero_ring(t):
                # one fat contiguous DMA: a column-wise ring DMA would emit
                # thousands of 2-byte descriptors and choke every DMA queue
                nc.sync.dma_start(t[:, :], z_d[:, :])

            def fwd_ops(src_tiles, dst_tiles, c):
                # V0 = r0-r2, V1 = r1+r2, V2 = r2-r1, V3 = r1-r3 where
                # r_a = src row 2t+a, for chunk c (tiles 7c..7c+6); thunks
                # ordered j-outer so the j=0 consumer matmul unblocks first.
                t0, t1 = TCH * c, TCH * (c + 1)
                ops = []
                for j in range(4):
                    for ci in range(2):
                        sv = view2(src_tiles[ci])
                        r0 = sv[:, t0:t1, 0, 1:1 + VW]
                        r1 = sv[:, t0:t1, 1, 1:1 + VW]
                        r2 = sv[:, t0 + 1:t1 + 1, 0, 1:1 + VW]
                        r3 = sv[:, t0 + 1:t1 + 1, 1, 1:1 + VW]
                        dst = vview(dst_tiles[(ci, c)])[:, j, :, :]
                        if j == 0:
                            ops.append(lambda e, d=dst, a=r0, b=r2: e.tensor_sub(d, a, b))
                        elif j == 1:
                            ops.append(lambda e, d=dst, a=r1, b=r2: e.tensor_add(d, a, b))
                        elif j == 2:
                            ops.append(lambda e, d=dst, a=r2, b=r1: e.tensor_sub(d, a, b))
                        else:
                            ops.append(lambda e, d=dst, a=r1, b=r3: e.tensor_sub(d, a, b))
                return ops

            def fwd_transform(eng, src_tiles, dst_tiles, c):
                for op in fwd_ops(src_tiles, dst_tiles, c):
                    op(eng)

            def mm_chunk(conv, v_tiles, co, c):
                # 24 matmuls accumulating M_j into PSUM banks j=0..3
                p = ps.tile([128, 4, 512], F32, name="pj")
                for j in range(4):
                    for kw in range(3):
                        for ci in range(2):
                            blk = (j * 3 + kw) * 2 + co
                            nc.tensor.matmul(
                                p[:, j, 0:NROW],
                                u_sb[(conv, ci)][:, blk * 128:(blk + 1) * 128],
                                vview(v_tiles[(ci, c)])[:, j, :, kw:kw + W],
                                start=(kw == 0 and ci == 0),
                                stop=(kw == 2 and ci == 1),
                            )
                return p

            def inverse(p, stt_eng=None):
                # y_even = M0+M1+M2 ; y_odd = M1-M2-M3 (from 4 PSUM banks)
                er = ep.tile([128, NROW], F32, name="er")
                t23 = ep.tile([128, NROW], F32, name="t23")
                orow = ep.tile([128, NROW], F32, name="orow")
                red = p[:, :, 0:NROW].rearrange("p j n -> p n j")
                nc.vector.tensor_reduce(er[:, :], red[:, :, 0:3], AxX, Alu.add)
                nc.vector.tensor_reduce(t23[:, :], red[:, :, 2:4], AxX, Alu.add)
                (stt_eng or nc.vector).scalar_tensor_tensor(
                    orow[:, :], p[:, 1, 0:NROW], 0.0, t23[:, :],
                    op0=Alu.bypass, op1=Alu.subtract)
                return er, orow

            def rows3(t):
                return t.rearrange("p (h w) -> p h w", h=TCH)

            # ---- startup staging ----
            # PE p-state warm-up: ~20 throwaway matmuls on scratch data keep
            # the PE busy through its clock ramp (0.65 -> 2.4 GHz over ~3us)
            # while the first DMAs land, so real matmuls start at full clock.
            scratch = wpool.tile([128, 512], F16, name="scratch")
            nc.sync.dma_start(scratch[:, :], z_d[:, 0:512])
            pw = ps.tile([128, 4, 512], F32, name="pj")
            for k in range(20):
                nc.tensor.matmul(
                    pw[:, k % 4, 0:NROW],
                    scratch[:, 0:128], scratch[:, 0:NROW],
                    start=True, stop=True)
            load_sample(0, bands=((0, 16),))
            load_weights(1, u1_d, blks=(0, 6))
            nc.sync.dma_start(b1_t[:, :], b1_d[:, :])
            load_weights(1, u1_d, blks=(6, 24))
            load_sample(0, bands=((16, 30), (30, 44), (44, PH)))
            # chunk-0 transform for sample 0 on DVE: its ops are ~3x cheaper
            # than GpSimd's and DVE is empty this early, so the first matmul
            # gates on ~2.5us of transforms instead of ~7us.
            fwd_transform(nc.vector,
                          {ci: xpad[(0, ci)] for ci in range(2)},
                          {(ci, 0): vx[(0, ci, 0)] for ci in range(2)}, 0)
            load_weights(2, u2_d)
            nc.sync.dma_start(b2_t[:, :], b2_d[:, :])
            for b in range(2):
                for ci in range(2):
                    zero_ring(o1pad[(b, ci)])
            load_sample(1)

            for s in range(S):
                b = s % 2
                b3 = s % 3

                # V(x, s0) chunks 1..3: queued 4 ops per conv1(s=0) co-slot
                s0_ops = []
                if s == 0:
                    for c in range(1, NCH):
                        s0_ops += fwd_ops(
                            {ci: xpad[(0, ci)] for ci in range(2)},
                            {(ci, cc): vx[(0, ci, cc)]
                             for ci in range(2) for cc in range(NCH)}, c)

                # prefetch transforms for the NEXT sample (x already resident,
                # loaded 2 samples ahead): cheap DVE ops, 2 per chunk slot of
                # BOTH conv1 and conv2 so DVE never saturates a single phase
                nxt_dve = []
                if s + 1 < S:
                    bn = (s + 1) % 2
                    xs = {ci: xpad[((s + 1) % 3, ci)] for ci in range(2)}
                    vs = {(ci, cc): vx[(bn, ci, cc)]
                          for ci in range(2) for cc in range(NCH)}
                    for cc in range(NCH):
                        nxt_dve += fwd_ops(xs, vs, cc)

                # conv1: x -> o1 (via vx), bias+relu on scalar into o1pad
                for c in range(NCH):
                    for co in range(2):
                        p = mm_chunk(
                            1, {(ci, cc): vx[(b, ci, cc)]
                                for ci in range(2) for cc in range(NCH)}, co, c)
                        er, orow = inverse(p)
                        ov = view2(o1pad[(b, co)])
                        # img even rows 2t -> pad row 2t+1; odd 2t+1 -> 2t+2
                        nc.scalar.activation(
                            ov[:, 7 * c:7 * c + 7, 1, 2:58], rows3(er), Relu,
                            bias=b1_t[:, co:co + 1])
                        nc.scalar.activation(
                            ov[:, 7 * c + 1:7 * c + 8, 0, 2:58], rows3(orow), Relu,
                            bias=b1_t[:, co:co + 1])
                        for _ in range(4):
                            if s0_ops:
                                s0_ops.pop(0)(nc.vector)
                        if not s0_ops:
                            for _ in range(2):
                                if nxt_dve:
                                    nxt_dve.pop(0)(nc.vector)
                    if c >= 1:
                        fwd_transform(nc.gpsimd,
                                      {ci: o1pad[(b, ci)] for ci in range(2)},
                                      vo, c - 1)
                fwd_transform(nc.gpsimd,
                              {ci: o1pad[(b, ci)] for ci in range(2)},
                              vo, 3)

                # conv2: o1 -> y (via vo), residual add on gpsimd, bias+relu scalar
                for c in range(NCH):
                    for co in range(2):
                        p = mm_chunk(2, vo, co, c)
                        er, orow = inverse(p)
                        xv = view2(xpad[(b3, co)])
                        ea = ep.tile([128, NROW], F32, name="ea")
                        oa = ep.tile([128, NROW], F32, name="oa")
                        nc.gpsimd.tensor_add(
                            rows3(ea), rows3(er),
                            xv[:, 7 * c:7 * c + 7, 1, 2:58])
                        nc.gpsimd.tensor_add(
                            rows3(oa), rows3(orow),
                            xv[:, 7 * c + 1:7 * c + 8, 0, 2:58])
                        ys = yp.tile([128, 2 * NROW], F32, name="ys")
                        yv = ys.rearrange("p (t two w) -> p t two w", two=2, w=W)
                        nc.scalar.activation(
                            yv[:, :, 0, :], rows3(ea), Relu,
                            bias=b2_t[:, co:co + 1])
                        nc.scalar.activation(
                            yv[:, :, 1, :], rows3(oa), Relu,
                            bias=b2_t[:, co:co + 1])
                        nc.sync.dma_start(
                            y_d[s, co * 128:(co + 1) * 128, 14 * c:14 * c + 14, :],
                            ys[:, :])
                        for _ in range(2):
                            if nxt_dve:
                                nxt_dve.pop(0)(nc.vector)
                while nxt_dve:
                    nxt_dve.pop(0)(nc.vector)

                if s + 2 < S:
                    load_sample(s + 2)

    nc.compile()
    return nc


def _get_nc():
    if "nc" not in _CACHE:
        _CACHE["nc"] = _build()
    return _CACHE["nc"]


G_WINO = np.array([[1, 0, 0], [0.5, 0.5, 0.5], [0.5, -0.5, 0.5], [0, 0, 1]],
                  dtype=np.float64)


def kernel(x, w1, g1, b1, m1, v1, w2, g2, b2, m2, v2):
    global LAST_RESULT
    from concourse import bass_utils

    x = np.asarray(x, dtype=np.float32)
    xp = np.zeros((x.shape[0], C, PH, PW), dtype=np.float16)
    xp[:, :, 1:57, 2:58] = x

    def fold(w, g, bb, m, v):
        inv = np.asarray(g, np.float64) / np.sqrt(np.asarray(v, np.float64) + EPS)
        wp = np.asarray(w, np.float64) * inv[:, None, None, None]
        bp = np.asarray(bb, np.float64) - np.asarray(m, np.float64) * inv
        # U[j, kw][ic, oc] = sum_kh G[j, kh] * wp[oc, ic, kh, kw]
        U = np.einsum('jk,oikw->jwio', G_WINO, wp)   # [4, 3, I, O]
        ut = np.zeros((2, 128, 24, 128), dtype=np.float16)
        for j in range(4):
            for kw in range(3):
                for co in range(2):
                    blk = (j * 3 + kw) * 2 + co
                    for ci in range(2):
                        ut[ci, :, blk, :] = U[j, kw, ci * 128:(ci + 1) * 128,
                                              co * 128:(co + 1) * 128]
        bt = np.ascontiguousarray(bp.reshape(2, 128).T).astype(np.float32)
        return ut, bt

    u1t, b1t = fold(w1, g1, b1, m1, v1)
    u2t, b2t = fold(w2, g2, b2, m2, v2)

    zeros = np.zeros((128, FLAT), dtype=np.float16)

    nc = _get_nc()
    in_maps = []
    for c in range(N_CORES):
        in_maps.append({
            "x": np.ascontiguousarray(xp[c * S:(c + 1) * S]),
            "u1t": u1t, "u2t": u2t, "b1t": b1t, "b2t": b2t,
            "zeros": zeros,
        })

    trace = bool(int(os.environ.get("BASS_KERNEL_TRACE", "0")))
    res = bass_utils.run_bass_kernel_spmd(
        nc, in_maps, core_ids=list(range(N_CORES)), trace=trace)
    LAST_RESULT = res
    out = np.concatenate([r["y"] for r in res.results], axis=0)
    return out


# revision 29
# speedup vs baseline: 1.0077x; 1.0077x over previous
"""ResNet BasicBlock (conv3x3-BN-ReLU-conv3x3-BN-add-ReLU) on 8 Trainium2 cores.

Data-parallel over batch: 32 samples -> 4 per core. Each 3x3 conv runs as a
Winograd F(2,3) transform along H (1.5x fewer PE MACs than direct conv):
rows are pre-combined on the GpSimd/Vector engines into 4 j-planes
(V0=r0-r2, V1=r1+r2, V2=r2-r1, V3=r1-r3, fp16), the PE accumulates
M_j = sum_{kw,ci} U_j[kw,ci]^T V_j(shifted kw) into 4 PSUM banks, and the
Vector engine folds the inverse transform (y_even = M0+M1+M2,
y_odd = M1-M2-M3) with one bank-spanning reduce + one scalar_tensor_tensor
per output row-pair chunk. BN scale is folded into the Winograd weights on
host; bias + ReLU run on the Scalar engine; the residual add runs on GpSimd.
V planes live in per-chunk tiles so Tile's whole-tile dependency tracking
stays chunk-granular (a matmul only waits for the transform wave it reads).
Images are zero-padded to 58x58 on the host so SBUF loads are contiguous.
"""
import os
import sys

for _p in ("/opt/trn_rl_repo", "/root/.axon_site/_ro/trn_rl_repo"):
    if os.path.isdir(_p) and _p not in sys.path:
        sys.path.append(_p)

import numpy as np

EPS = 1e-5

S = 4            # samples per core
C = 256
H = W = 56
PH = 58          # padded rows (img rows -1..56)
PW = 60          # row pitch: image cols at 2..57, ring zeros at cols 1,58
FLAT = PH * PW   # 3480
T = 28           # winograd row-tiles per image (2 output rows each)
TCH = 7          # tiles per chunk
NCH = 4          # chunks (4*7 = 28 tiles)
NROW = TCH * W   # 392 moving rows per matmul
VW = 58          # V plane cols (xpad cols 1..58)
VCH = 4 * TCH * VW  # V chunk-tile free size: [4j, 7t, 58]
N_CORES = 8

_CACHE = {}
LAST_RESULT = None


def _build():
    from concourse import bacc
    import concourse.mybir as mybir
    import concourse.tile as tile

    F32 = mybir.dt.float32
    F16 = mybir.dt.float16
    Relu = mybir.ActivationFunctionType.Relu
    Alu = mybir.AluOpType
    AxX = mybir.AxisListType.X

    nc = bacc.Bacc(None, target_bir_lowering=False)

    x_d = nc.dram_tensor("x", [S, C, PH, PW], F16, kind="ExternalInput")
    u1_d = nc.dram_tensor("u1t", [2, 128, 24, 128], F16, kind="ExternalInput")
    u2_d = nc.dram_tensor("u2t", [2, 128, 24, 128], F16, kind="ExternalInput")
    b1_d = nc.dram_tensor("b1t", [128, 2], F32, kind="ExternalInput")
    b2_d = nc.dram_tensor("b2t", [128, 2], F32, kind="ExternalInput")
    z_d = nc.dram_tensor("zeros", [128, FLAT], F16, kind="ExternalInput")
    y_d = nc.dram_tensor("y", [S, C, H, W], F32, kind="ExternalOutput")

    with tile.TileContext(nc) as tc:
        with (
            tc.tile_pool(name="wpool", bufs=1) as wpool,
            tc.tile_pool(name="img", bufs=1) as img,
            tc.tile_pool(name="ep", bufs=3) as ep,
            tc.tile_pool(name="yp", bufs=3) as yp,
            tc.tile_pool(name="ps", bufs=2, space="PSUM") as ps,
        ):
            u_sb = {}
            for conv in (1, 2):
                for ci in range(2):
                    u_sb[(conv, ci)] = wpool.tile(
                        [128, 24 * 128], F16, name=f"u{conv}_{ci}")
            b1_t = wpool.tile([128, 2], F32, name="b1_t")
            b2_t = wpool.tile([128, 2], F32, name="b2_t")

            xpad = {}
            o1pad = {}
            vx = {}   # (b, ci, c) -> [128, 4, 7, 58] chunk tile
            vo = {}   # (ci, c)    -> [128, 4, 7, 58] chunk tile
            # xpad is triple-buffered (keyed s%3): load_sample(s+2) must not
            # WAR-depend on conv2(s)'s residual reads, or the scheduler's
            # hoisted V(x) transforms deadlock the DVE queue against it.
            for b in range(3):
                for ci in range(2):
                    xpad[(b, ci)] = img.tile([128, FLAT], F16, name=f"xpad{b}_{ci}")
            for b in range(2):
                for ci in range(2):
                    o1pad[(b, ci)] = img.tile([128, FLAT], F16, name=f"o1pad{b}_{ci}")
                    for c in range(NCH):
                        vx[(b, ci, c)] = img.tile(
                            [128, VCH], F16, name=f"vx{b}_{ci}_{c}")
            for ci in range(2):
                for c in range(NCH):
                    vo[(ci, c)] = img.tile([128, VCH], F16, name=f"vo_{ci}_{c}")

            def view(t):
                return t.rearrange("p (h w) -> p h w", h=PH)

            def view2(t):
                # [p, 29, 2, 60]: row 2t+a at [:, t, a, :]
                return t.rearrange("p (t two w) -> p t two w", two=2, w=PW)

            def vview(t):
                return t.rearrange("p (j t w) -> p j t w", j=4, t=TCH)

            def load_weights(conv, ud, blks=(0, 24)):
                k0, k1 = blks
                for ci in range(2):
                    nc.sync.dma_start(
                        u_sb[(conv, ci)][:, k0 * 128:k1 * 128],
                        ud[ci, :, k0:k1, :].rearrange("p a b -> p (a b)"))

            def load_sample(s, bands=((0, PH),)):
                b = s % 3
                for r0, r1 in bands:
                    for ci in range(2):
                        nc.sync.dma_start(
                            view(xpad[(b, ci)])[:, r0:r1, :],
                            x_d[s, ci * 128:(ci + 1) * 128, r0:r1, :])

            def zero_ring(t):
                # one fat contiguous DMA: a column-wise ring DMA would emit
                # thousands of 2-byte descriptors and choke every DMA queue
                nc.sync.dma_start(t[:, :], z_d[:, :])

            def fwd_ops(src_tiles, dst_tiles, c):
                # V0 = r0-r2, V1 = r1+r2, V2 = r2-r1, V3 = r1-r3 where
                # r_a = src row 2t+a, for chunk c (tiles 7c..7c+6); thunks
                # ordered j-outer so the j=0 consumer matmul unblocks first.
                t0, t1 = TCH * c, TCH * (c + 1)
                ops = []
                for j in range(4):
                    for ci in range(2):
                        sv = view2(src_tiles[ci])
                        r0 = sv[:, t0:t1, 0, 1:1 + VW]
                        r1 = sv[:, t0:t1, 1, 1:1 + VW]
                        r2 = sv[:, t0 + 1:t1 + 1, 0, 1:1 + VW]
                        r3 = sv[:, t0 + 1:t1 + 1, 1, 1:1 + VW]
                        dst = vview(dst_tiles[(ci, c)])[:, j, :, :]
                        if j == 0:
                            ops.append(lambda e, d=dst, a=r0, b=r2: e.tensor_sub(d, a, b))
                        elif j == 1:
                            ops.append(lambda e, d=dst, a=r1, b=r2: e.tensor_add(d, a, b))
                        elif j == 2:
                            ops.append(lambda e, d=dst, a=r2, b=r1: e.tensor_sub(d, a, b))
                        else:
                            ops.append(lambda e, d=dst, a=r1, b=r3: e.tensor_sub(d, a, b))
                return ops

            def fwd_transform(eng, src_tiles, dst_tiles, c):
                for op in fwd_ops(src_tiles, dst_tiles, c):
                    op(eng)

            def mm_chunk(conv, v_tiles, co, c):
                # 24 matmuls accumulating M_j into PSUM banks j=0..3
                p = ps.tile([128, 4, 512], F32, name="pj")
                for j in range(4):
                    for kw in range(3):
                        for ci in range(2):
                            blk = (j * 3 + kw) * 2 + co
                            nc.tensor.matmul(
                                p[:, j, 0:NROW],
                                u_sb[(conv, ci)][:, blk * 128:(blk + 1) * 128],
                                vview(v_tiles[(ci, c)])[:, j, :, kw:kw + W],
                                start=(kw == 0 and ci == 0),
                                stop=(kw == 2 and ci == 1),
                            )
                return p

            def inverse(p, stt_eng=None):
                # y_even = M0+M1+M2 ; y_odd = M1-M2-M3 (from 4 PSUM banks)
                er = ep.tile([128, NROW], F32, name="er")
                t23 = ep.tile([128, NROW], F32, name="t23")
                orow = ep.tile([128, NROW], F32, name="orow")
                red = p[:, :, 0:NROW].rearrange("p j n -> p n j")
                nc.vector.tensor_reduce(er[:, :], red[:, :, 0:3], AxX, Alu.add)
                nc.vector.tensor_reduce(t23[:, :], red[:, :, 2:4], AxX, Alu.add)
                (stt_eng or nc.vector).scalar_tensor_tensor(
                    orow[:, :], p[:, 1, 0:NROW], 0.0, t23[:, :],
                    op0=Alu.bypass, op1=Alu.subtract)
                return er, orow

            def rows3(t):
                return t.rearrange("p (h w) -> p h w", h=TCH)

            # ---- startup staging ----
            load_weights(1, u1_d, blks=(0, 6))
            nc.sync.dma_start(b1_t[:, :], b1_d[:, :])
            load_sample(0, bands=((0, 16),))
            load_weights(1, u1_d, blks=(6, 24))
            load_sample(0, bands=((16, 30), (30, 44), (44, PH)))
            # chunk-0 transform for sample 0 on DVE: its ops are ~3x cheaper
            # than GpSimd's and DVE is empty this early, so the first matmul
            # gates on ~2.5us of transforms instead of ~7us.
            fwd_transform(nc.vector,
                          {ci: xpad[(0, ci)] for ci in range(2)},
                          {(ci, 0): vx[(0, ci, 0)] for ci in range(2)}, 0)
            load_weights(2, u2_d)
            nc.sync.dma_start(b2_t[:, :], b2_d[:, :])
            for b in range(2):
                for ci in range(2):
                    zero_ring(o1pad[(b, ci)])
            load_sample(1)

            for s in range(S):
                b = s % 2
                b3 = s % 3

                # V(x, s0) chunks 1..3: queued 4 ops per conv1(s=0) co-slot
                s0_ops = []
                if s == 0:
                    for c in range(1, NCH):
                        s0_ops += fwd_ops(
                            {ci: xpad[(0, ci)] for ci in range(2)},
                            {(ci, cc): vx[(0, ci, cc)]
                             for ci in range(2) for cc in range(NCH)}, c)

                # prefetch transforms for the NEXT sample (x already resident,
                # loaded 2 samples ahead): cheap DVE ops, 2 per chunk slot of
                # BOTH conv1 and conv2 so DVE never saturates a single phase
                nxt_dve = []
                if s + 1 < S:
                    bn = (s + 1) % 2
                    xs = {ci: xpad[((s + 1) % 3, ci)] for ci in range(2)}
                    vs = {(ci, cc): vx[(bn, ci, cc)]
                          for ci in range(2) for cc in range(NCH)}
                    for cc in range(NCH):
                        nxt_dve += fwd_ops(xs, vs, cc)

                # conv1: x -> o1 (via vx), bias+relu on scalar into o1pad
                for c in range(NCH):
                    for co in range(2):
                        p = mm_chunk(
                            1, {(ci, cc): vx[(b, ci, cc)]
                                for ci in range(2) for cc in range(NCH)}, co, c)
                        er, orow = inverse(p)
                        ov = view2(o1pad[(b, co)])
                        # img even rows 2t -> pad row 2t+1; odd 2t+1 -> 2t+2
                        nc.scalar.activation(
                            ov[:, 7 * c:7 * c + 7, 1, 2:58], rows3(er), Relu,
                            bias=b1_t[:, co:co + 1])
                        nc.scalar.activation(
                            ov[:, 7 * c + 1:7 * c + 8, 0, 2:58], rows3(orow), Relu,
                            bias=b1_t[:, co:co + 1])
                        for _ in range(4):
                            if s0_ops:
                                s0_ops.pop(0)(nc.vector)
                        if not s0_ops:
                            for _ in range(2):
                                if nxt_dve:
                                    nxt_dve.pop(0)(nc.vector)
                    if c >= 1:
                        fwd_transform(nc.gpsimd,
                                      {ci: o1pad[(b, ci)] for ci in range(2)},
                                      vo, c - 1)
                fwd_transform(nc.gpsimd,
                              {ci: o1pad[(b, ci)] for ci in range(2)},
                              vo, 3)

                # conv2: o1 -> y (via vo), residual add on gpsimd, bias+relu scalar
                for c in range(NCH):
                    for co in range(2):
                        p = mm_chunk(2, vo, co, c)
                        er, orow = inverse(p)
                        xv = view2(xpad[(b3, co)])
                        ea = ep.tile([128, NROW], F32, name="ea")
                        oa = ep.tile([128, NROW], F32, name="oa")
                        nc.gpsimd.tensor_add(
                            rows3(ea), rows3(er),
                            xv[:, 7 * c:7 * c + 7, 1, 2:58])
                        nc.gpsimd.tensor_add(
                            rows3(oa), rows3(orow),
                            xv[:, 7 * c + 1:7 * c + 8, 0, 2:58])
                        ys = yp.tile([128, 2 * NROW], F32, name="ys")
                        yv = ys.rearrange("p (t two w) -> p t two w", two=2, w=W)
                        nc.scalar.activation(
                            yv[:, :, 0, :], rows3(ea), Relu,
                            bias=b2_t[:, co:co + 1])
                        nc.scalar.activation(
                            yv[:, :, 1, :], rows3(oa), Relu,
                            bias=b2_t[:, co:co + 1])
                        nc.sync.dma_start(
                            y_d[s, co * 128:(co + 1) * 128, 14 * c:14 * c + 14, :],
                            ys[:, :])
                        for _ in range(2):
                            if nxt_dve:
                                nxt_dve.pop(0)(nc.vector)
                while nxt_dve:
                    nxt_dve.pop(0)(nc.vector)

                if s + 2 < S:
                    load_sample(s + 2)

    nc.compile()
    return nc


def _get_nc():
    if "nc" not in _CACHE:
        _CACHE["nc"] = _build()
    return _CACHE["nc"]


G_WINO = np.array([[1, 0, 0], [0.5, 0.5, 0.5], [0.5, -0.5, 0.5], [0, 0, 1]],
                  dtype=np.float64)


def kernel(x, w1, g1, b1, m1, v1, w2, g2, b2, m2, v2):
    global LAST_RESULT
    from concourse import bass_utils

    x = np.asarray(x, dtype=np.float32)
    xp = np.zeros((x.shape[0], C, PH, PW), dtype=np.float16)
    xp[:, :, 1:57, 2:58] = x

    def fold(w, g, bb, m, v):
        inv = np.asarray(g, np.float64) / np.sqrt(np.asarray(v, np.float64) + EPS)
        wp = np.asarray(w, np.float64) * inv[:, None, None, None]
        bp = np.asarray(bb, np.float64) - np.asarray(m, np.float64) * inv
        # U[j, kw][ic, oc] = sum_kh G[j, kh] * wp[oc, ic, kh, kw]
        U = np.einsum('jk,oikw->jwio', G_WINO, wp)   # [4, 3, I, O]
        ut = np.zeros((2, 128, 24, 128), dtype=np.float16)
        for j in range(4):
            for kw in range(3):
                for co in range(2):
                    blk = (j * 3 + kw) * 2 + co
                    for ci in range(2):
                        ut[ci, :, blk, :] = U[j, kw, ci * 128:(ci + 1) * 128,
                                              co * 128:(co + 1) * 128]
        bt = np.ascontiguousarray(bp.reshape(2, 128).T).astype(np.float32)
        return ut, bt

    u1t, b1t = fold(w1, g1, b1, m1, v1)
    u2t, b2t = fold(w2, g2, b2, m2, v2)

    zeros = np.zeros((128, FLAT), dtype=np.float16)

    nc = _get_nc()
    in_maps = []
    for c in range(N_CORES):
        in_maps.append({
            "x": np.ascontiguousarray(xp[c * S:(c + 1) * S]),
            "u1t": u1t, "u2t": u2t, "b1t": b1t, "b2t": b2t,
            "zeros": zeros,
        })

    trace = bool(int(os.environ.get("BASS_KERNEL_TRACE", "0")))
    res = bass_utils.run_bass_kernel_spmd(
        nc, in_maps, core_ids=list(range(N_CORES)), trace=trace)
    LAST_RESULT = res
    out = np.concatenate([r["y"] for r in res.results], axis=0)
    return out
